# revision 15
# baseline (speedup 1.0000x reference)
"""GCN 2-layer kernel for TRN2, 8 cores, gather-scatter via dma_gather +
selection-matrix matmuls.

Structure (per core, dst-sharded):
  L1: aggregate x over edges into aggT (feature-major), interleaved with
      dense (W1+relu, W2) and per-slice AllGather of m = h@W2.
  L2: aggregate m over edges, in quarter-major passes where quarter q of the
      gather table is the slice-q AllGather output tensor (per-slice tensors
      give exact AG->gather deps), accumulated into agg2T in SBUF.

Key constraints baked in:
  - dma_gather with single_packet=True: at most 8 chunks (1024 idxs) per
    call (64-descriptor packet ceiling per SDMA lane).
  - gather idx is int16: tables are split into quarters < 32768 rows.
  - gather rows must be a multiple of 256 bytes.

cfg knobs: elem1/elem2 (table row width in fp16 elems), wg (windows per
dense group), max_call_chunks, wps (list of windows per AG slice),
single_packet, diag (self-loops via contiguous DMA), do_l1/do_ag/do_l2.
"""
import numpy as np

import concourse.bacc as bacc
import concourse.mybir as mybir
from concourse import tile
from concourse.bass_utils import run_bass_kernel_spmd

N_NODES = 100000
IN_DIM, HID_DIM, OUT_DIM = 128, 128, 64
N_CORES = 8
SHARD = N_NODES // N_CORES          # 12500
WIN = 128
N_WIN = (SHARD + WIN - 1) // WIN    # 98
SHARD_PAD = N_WIN * WIN             # 12544
DT16 = mybir.dt.float16
DT32 = mybir.dt.float32


def _plan_layer(dstl_c, q_c, idx_c, norm_c, wg, max_call_chunks,
                nq_real, passes, diag_nm_c=None):
    """Build the chunk/call/wg schedule for one aggregation layer.

    dstl_c/q_c/idx_c/norm_c: per-core edge arrays (local dst, quarter,
    idx-within-quarter, norm weight). passes: list of quarter-lists; chunks
    are laid out pass-major, then window-group, then quarter, then window.
    diag_nm_c: optional per-core [SHARD_PAD] self-loop norms -> one extra
    'diag' chunk per window (pseudo-quarter nq_real, filled by contiguous
    DMA), assigned to the first pass.
    """
    nq = nq_real + 1 if diag_nm_c is not None else nq_real
    n_cells = N_WIN * nq
    per_core = []
    counts = np.zeros((N_CORES, n_cells), np.int64)
    for c in range(N_CORES):
        dstl, q, idx, norm = dstl_c[c], q_c[c], idx_c[c], norm_c[c]
        win = dstl // WIN
        g = win // wg
        order = np.lexsort((idx, win, q, g))
        dstl, q, idx, norm, win = (dstl[order], q[order], idx[order],
                                   norm[order], win[order])
        cell = win * nq + q
        counts[c] = np.bincount(cell, minlength=n_cells)
        per_core.append((dstl, q, idx, norm, cell))
    if diag_nm_c is not None:
        counts[:, np.arange(N_WIN) * nq + nq_real] = 128
    cell_chunks = np.maximum(np.ceil(counts.max(axis=0) / 128).astype(np.int64), 1)

    n_wg = (N_WIN + wg - 1) // wg
    cell_order = []
    wg_specs = []  # (pass_id, wins, [cells in chunk order])
    for pi, pq in enumerate(passes):
        qlist = list(pq) + ([nq_real] if (diag_nm_c is not None and pi == 0)
                            else [])
        for g in range(n_wg):
            wins = list(range(g * wg, min((g + 1) * wg, N_WIN)))
            cls = [w * nq + q for q in qlist for w in wins]
            cell_order.extend(cls)
            wg_specs.append((pi, wins, cls))
    cell_order = np.array(cell_order)
    assert len(set(cell_order.tolist())) == len(cell_order)

    cell_chunk_start = np.zeros(n_cells, np.int64)
    acc = 0
    for cl in cell_order:
        cell_chunk_start[cl] = acc
        acc += cell_chunks[cl]
    total_chunks = int(acc)
    total_slots = total_chunks * 128

    chunk_win = np.zeros(total_chunks, np.int64)
    chunk_q = np.zeros(total_chunks, np.int64)
    for cl in range(n_cells):
        s = cell_chunk_start[cl]
        for k in range(cell_chunks[cl]):
            chunk_win[s + k] = cl // nq
            chunk_q[s + k] = cl % nq

    calls = []
    i = 0
    while i < total_chunks:
        if chunk_q[i] == nq_real:
            i += 1
            continue
        j = i
        while (j < total_chunks and chunk_q[j] == chunk_q[i]
               and j - i < max_call_chunks):
            j += 1
        calls.append((int(chunk_q[i]), int(i), int(j - i)))
        i = j

    idx16 = np.zeros((N_CORES, total_slots), np.int16)
    dn = np.zeros((N_CORES, total_slots), np.float32)
    nm = np.zeros((N_CORES, total_slots), np.float32)
    for c in range(N_CORES):
        dstl, q, idx, norm, cell = per_core[c]
        cnt = counts[c]
        edge_off = np.zeros(n_cells, np.int64)
        pos = 0
        for cl in cell_order:
            if cl % nq == nq_real:
                continue
            edge_off[cl] = pos
            pos += cnt[cl]
        for cl in range(n_cells):
            if cl % nq == nq_real:
                w = cl // nq
                s = int(cell_chunk_start[cl]) * 128
                dn[c, s:s + 128] = np.arange(128, dtype=np.float32)
                nm[c, s:s + 128] = diag_nm_c[c][w * 128:(w + 1) * 128]
                continue
            n_e = int(cnt[cl])
            s = int(cell_chunk_start[cl]) * 128
            eo = int(edge_off[cl])
            idx16[c, s:s + n_e] = idx[eo:eo + n_e].astype(np.int16)
            dn[c, s:s + n_e] = (dstl[eo:eo + n_e] % WIN).astype(np.float32)
            nm[c, s:s + n_e] = norm[eo:eo + n_e]

    total_cols = total_slots // 16
    idx_packed = np.zeros((N_CORES, 128, total_cols), np.int16)
    for c in range(N_CORES):
        t = idx16[c].reshape(total_slots // 16, 16).T
        idx_packed[c] = np.tile(t, (8, 1))
    dn_t = dn.reshape(N_CORES, total_chunks, 128).transpose(0, 2, 1).copy()
    nm_t = nm.reshape(N_CORES, total_chunks, 128).transpose(0, 2, 1).copy()

    wgs = []
    for (pi, wins, cls) in wg_specs:
        lo = min(cell_chunk_start[cl] for cl in cls)
        hi = max(cell_chunk_start[cl] + cell_chunks[cl] for cl in cls)
        call_ids = [i for i, (q, s, n) in enumerate(calls) if lo <= s < hi]
        wgs.append((pi, wins, int(lo), int(hi), call_ids))

    return dict(
        total_chunks=total_chunks, calls=calls, wgs=wgs,
        chunk_win=chunk_win, chunk_q=chunk_q,
        idx_packed=idx_packed, dn=dn_t, nm=nm_t,
        max_wg_chunks=max(hi - lo for (_, _, lo, hi, _) in wgs),
        has_diag=diag_nm_c is not None, nq_real=nq_real,
        n_passes=len(passes),
    )


_PLAN_CACHE = {}


def _build_plans(edge_index, wg, max_call_chunks, wps, diag=True,
                 l2_passes=True):
    key = (wg, max_call_chunks, tuple(wps), diag, l2_passes)
    if key in _PLAN_CACHE:
        return _PLAN_CACHE[key]
    src0 = np.asarray(edge_index[0], dtype=np.int64)
    dst0 = np.asarray(edge_index[1], dtype=np.int64)
    loops = np.arange(N_NODES, dtype=np.int64)
    deg = np.bincount(dst0, minlength=N_NODES).astype(np.float64) + 1.0
    dinv = 1.0 / np.sqrt(deg)
    diag_nm_c = np.zeros((N_CORES, SHARD_PAD), np.float32)
    for c in range(N_CORES):
        diag_nm_c[c, :SHARD] = (dinv[c * SHARD:(c + 1) * SHARD] ** 2)

    def shard_edges(with_loops):
        if with_loops:
            src = np.concatenate([src0, loops])
            dst = np.concatenate([dst0, loops])
        else:
            src, dst = src0, dst0
        norm = (dinv[src] * dinv[dst]).astype(np.float32)
        owner = dst // SHARD
        dstl_c, src_c, norm_c = [], [], []
        for c in range(N_CORES):
            m = owner == c
            dstl_c.append((dst[m] - c * SHARD).astype(np.int64))
            src_c.append(src[m])
            norm_c.append(norm[m])
        return dstl_c, src_c, norm_c

    dstl_c, src_c, norm_c = shard_edges(not diag)

    # Layer 1: per-core dedup'd sorted table, 3 quarters (idx < 32768).
    uniq_c, q1_c, idx1_c = [], [], []
    maxu = 0
    for c in range(N_CORES):
        u, inv = np.unique(src_c[c], return_inverse=True)
        uniq_c.append(u)
        q1_c.append(inv)  # placeholder, fixed below once qsize known
        idx1_c.append(inv)
        maxu = max(maxu, len(u))
    qs1 = -(-maxu // 3)
    assert qs1 <= 32768
    q1_c = [inv // qs1 for inv in idx1_c]
    idx1_c = [inv % qs1 for inv in idx1_c]
    plan1 = _plan_layer(dstl_c, q1_c, idx1_c, norm_c, wg, max_call_chunks,
                        nq_real=3, passes=[[0, 1, 2]],
                        diag_nm_c=diag_nm_c if diag else None)
    plan1["uniq_c"] = uniq_c
    plan1["uniq_pad"] = 3 * qs1

    # Layer 2: quarter q == AG slice q (per-slice m tensors).
    n_sl = len(wps)
    slice_start_win = np.concatenate([[0], np.cumsum(wps)])[:-1]
    assert sum(wps) == N_WIN
    slice_of_win = np.repeat(np.arange(n_sl), wps)
    rpc = [w * 128 for w in wps]  # rows per core per slice
    assert all(r * N_CORES <= 32768 for r in rpc)
    q2_c, idx2_c = [], []
    for c in range(N_CORES):
        s = src_c[c]
        c_own = s // SHARD
        l = s % SHARD
        wl = l // 128
        sl = slice_of_win[wl]
        idx = (c_own * np.array(rpc)[sl]
               + (l - slice_start_win[sl] * 128))
        q2_c.append(sl.astype(np.int64))
        idx2_c.append(idx.astype(np.int64))
    passes2 = [[q] for q in range(n_sl)] if l2_passes else [list(range(n_sl))]
    plan2 = _plan_layer(dstl_c, q2_c, idx2_c, norm_c, wg, max_call_chunks,
                        nq_real=n_sl, passes=passes2,
                        diag_nm_c=diag_nm_c if diag else None)
    _PLAN_CACHE[key] = (plan1, plan2)
    return plan1, plan2


def _emit_agg_phase(nc, tc, plan, tables, idx_d, dn_d, nm_d, iota_sb,
                    f_out, out_cb, tag, elem, cfg, diag_d=None):
    """tables: list of DRAM APs, one per quarter. out_cb(w, psum, pass_id)."""
    calls = plan["calls"]
    nchunks = plan["total_chunks"]
    sp_flag = cfg.get("single_packet", True)
    with (
        tc.tile_pool(name=f"idx{tag}", bufs=3) as idxp,
        tc.tile_pool(name=f"msg{tag}", bufs=2) as msgp,
        tc.tile_pool(name=f"s{tag}", bufs=8) as sp,
        tc.tile_pool(name=f"dnm{tag}", bufs=1) as dnmp,
        tc.tile_pool(name=f"ps{tag}", bufs=4, space="PSUM") as psp,
    ):
        dn_sb = dnmp.tile([128, nchunks], DT32)
        nc.sync.dma_start(dn_sb[:], dn_d[:])
        nm_sb = dnmp.tile([128, nchunks], DT32)
        nc.sync.dma_start(nm_sb[:], nm_d[:])
        qn = 0
        for (pi, wins, lo, hi, call_ids) in plan["wgs"]:
            nch = hi - lo
            msgs = msgp.tile([128, plan["max_wg_chunks"], elem], DT16)
            it = idxp.tile([128, plan["max_wg_chunks"] * 8], mybir.dt.int16)
            nc.sync.dma_start(it[:, :nch * 8], idx_d[:, lo * 8: hi * 8])
            for ci in call_ids:
                (q, s, n) = calls[ci]
                nidx = n * 128
                cols = nidx // 16
                nc.gpsimd.dma_gather(
                    msgs[:, s - lo: s - lo + n, :],
                    tables[q],
                    it[:, (s - lo) * 8: (s - lo) * 8 + cols],
                    nidx, nidx, elem, queue_num=qn % 4,
                    single_packet=sp_flag)
                qn += 1
            if plan.get("has_diag"):
                for k in range(lo, hi):
                    if int(plan["chunk_q"][k]) == plan["nq_real"]:
                        w = int(plan["chunk_win"][k])
                        nc.sync.dma_start(
                            msgs[:, k - lo, :],
                            diag_d[w * 128:(w + 1) * 128, 0:elem])
            psums = {}
            first_of_win, last_of_win = {}, {}
            for k in range(lo, hi):
                w = int(plan["chunk_win"][k])
                first_of_win.setdefault(w, k)
                last_of_win[w] = k
            for k in range(lo, hi):
                w = int(plan["chunk_win"][k])
                st = sp.tile([128, WIN], DT16)
                nc.vector.tensor_scalar(
                    out=st[:], in0=iota_sb[:],
                    scalar1=dn_sb[:, k:k + 1], scalar2=nm_sb[:, k:k + 1],
                    op0=mybir.AluOpType.is_equal, op1=mybir.AluOpType.mult)
                if w not in psums:
                    psums[w] = psp.tile([f_out, WIN], DT32, name=f"psw{tag}",
                                        tag=f"psw{tag}")
                nc.tensor.matmul(
                    psums[w][:], lhsT=msgs[:, k - lo, 0:f_out], rhs=st[:],
                    start=(k == first_of_win[w]),
                    stop=(k == last_of_win[w]))
            for w in wins:
                out_cb(w, psums[w], pi)


DEFAULT_CFG = {}


def build_kernel(edge_index, w1, b1, w2, b2, x, reps=1, cfg=None):
    cfg = dict(DEFAULT_CFG if cfg is None else cfg)
    elem1 = cfg.get("elem1", 256)
    elem2 = cfg.get("elem2", 128)
    wg = cfg.get("wg", 3)
    mcc = cfg.get("max_call_chunks", 8)
    wps = cfg.get("wps", [30, 30, 30, 8])
    if isinstance(wps, int):
        n_even = N_WIN // wps
        wps = [wps] * n_even + ([N_WIN - wps * n_even]
                                if N_WIN % wps else [])
    diag = cfg.get("diag", True)
    l2_passes = cfg.get("l2_passes", True)
    do_l1 = cfg.get("do_l1", True)
    do_ag = cfg.get("do_ag", True)
    do_l2 = cfg.get("do_l2", True)
    for e in np.cumsum(wps)[:-1]:
        assert e % wg == 0, (wps, wg)

    plan1, plan2 = _build_plans(edge_index, wg, mcc, wps, diag=diag,
                                l2_passes=l2_passes)
    n_sl = len(wps)
    rpc = [w * 128 for w in wps]
    slice_start_win = np.concatenate([[0], np.cumsum(wps)])[:-1]
    slice_ends = set(int(e) for e in np.cumsum(wps))

    nc = bacc.Bacc("TRN2", num_devices=N_CORES, num_swdge_queues=4)
    n1c = plan1["total_chunks"]
    n2c = plan2["total_chunks"]
    uniq_pad = plan1["uniq_pad"]
    xt_d = nc.dram_tensor("xt", [uniq_pad, elem1], DT16, kind="ExternalInput")
    idx1_d = nc.dram_tensor("idx1", [128, n1c * 8], mybir.dt.int16, kind="ExternalInput")
    idx2_d = nc.dram_tensor("idx2", [128, n2c * 8], mybir.dt.int16, kind="ExternalInput")
    dn1_d = nc.dram_tensor("dn1", [128, n1c], DT32, kind="ExternalInput")
    nm1_d = nc.dram_tensor("nm1", [128, n1c], DT32, kind="ExternalInput")
    dn2_d = nc.dram_tensor("dn2", [128, n2c], DT32, kind="ExternalInput")
    nm2_d = nc.dram_tensor("nm2", [128, n2c], DT32, kind="ExternalInput")
    w1_d = nc.dram_tensor("w1", [128, HID_DIM], DT16, kind="ExternalInput")
    w2_d = nc.dram_tensor("w2", [128, OUT_DIM], DT16, kind="ExternalInput")
    b1_d = nc.dram_tensor("b1", [128, 1], DT32, kind="ExternalInput")
    b2_d = nc.dram_tensor("b2", [OUT_DIM, 1], DT32, kind="ExternalInput")
    iota_d = nc.dram_tensor("iota", [128, WIN], DT16, kind="ExternalInput")
    xdiag_d = (nc.dram_tensor("xdiag", [SHARD_PAD, elem1], DT16,
                              kind="ExternalInput") if diag else None)
    out_d = nc.dram_tensor("outT", [OUT_DIM, SHARD_PAD], DT32, kind="ExternalOutput")
    m_local = nc.dram_tensor("m_local", [SHARD_PAD, elem2], DT16, kind="Internal")
    m_sl = [nc.dram_tensor(f"m_sl{s}", [N_CORES * rpc[s], elem2], DT16,
                           kind="Internal", addr_space="Shared")
            for s in range(n_sl)]

    qs1 = uniq_pad // 3
    tables1 = [xt_d[q * qs1:(q + 1) * qs1, :] for q in range(3)]
    tables2 = [m_sl[s][:, :] for s in range(n_sl)]

    with tile.TileContext(nc) as tc:
      for _rep in range(reps):
        with (
            tc.tile_pool(name="persist", bufs=1) as pp,
            tc.tile_pool(name="mtile", bufs=4) as mp,
            tc.tile_pool(name="dps", bufs=2, space="PSUM") as dps,
        ):
            iota_sb = pp.tile([128, WIN], DT16)
            nc.sync.dma_start(iota_sb[:], iota_d[:])
            w1_sb = pp.tile([128, HID_DIM], DT16)
            nc.sync.dma_start(w1_sb[:], w1_d[:])
            w2_sb = pp.tile([128, OUT_DIM], DT16)
            nc.sync.dma_start(w2_sb[:], w2_d[:])
            b1_sb = pp.tile([128, 1], DT32)
            nc.sync.dma_start(b1_sb[:], b1_d[:])
            b2_sb = pp.tile([OUT_DIM, 1], DT32)
            nc.sync.dma_start(b2_sb[:], b2_d[:])
            aggT = pp.tile([128, SHARD_PAD], DT16)
            hT = pp.tile([128, SHARD_PAD], DT16)

            def dense_for_windows(w0, w1):
                """dense1+dense2+m write for windows [w0, w1)."""
                cols = (w1 - w0) * WIN
                t = w0 * WIN
                ph = dps.tile([128, 512], DT32)
                nc.tensor.matmul(ph[:, :cols], lhsT=w1_sb[:],
                                 rhs=aggT[:, t:t + cols], start=True, stop=True)
                nc.scalar.activation(hT[:, t:t + cols], ph[:, :cols],
                                     mybir.ActivationFunctionType.Relu,
                                     bias=b1_sb[:, 0:1], scale=1.0)
                for w in range(w0, w1):
                    pm = dps.tile([128, OUT_DIM], DT32)
                    nc.tensor.matmul(pm[:], lhsT=hT[:, w * 128:(w + 1) * 128],
                                     rhs=w2_sb[:], start=True, stop=True)
                    msb = mp.tile([128, elem2], DT16)
                    nc.vector.tensor_copy(msb[:, 0:OUT_DIM], pm[:])
                    if elem2 > OUT_DIM:
                        nc.vector.memset(msb[:, OUT_DIM:elem2], 0.0)
                    nc.sync.dma_start(m_local[w * 128:(w + 1) * 128, :], msb[:])

            def ag_slice(sl):
                r0 = int(slice_start_win[sl]) * 128
                nc.gpsimd.collective_compute(
                    "AllGather", mybir.AluOpType.bypass,
                    replica_groups=[list(range(N_CORES))],
                    ins=[m_local[r0:r0 + rpc[sl], :]],
                    outs=[m_sl[sl][:, :]])

            if do_l1:
                def l1_out(w, ps, pi):
                    nc.vector.tensor_copy(aggT[:, w * WIN:(w + 1) * WIN], ps[:])
                    if (w + 1) % wg == 0 or w == N_WIN - 1:
                        dense_for_windows((w // wg) * wg, w + 1)
                    if do_ag and ((w + 1) in slice_ends):
                        ag_slice(int(np.searchsorted(np.cumsum(wps), w + 1)))
                _emit_agg_phase(nc, tc, plan1, tables1, idx1_d, dn1_d, nm1_d,
                                iota_sb, IN_DIM, l1_out, "g1", elem1, cfg,
                                diag_d=xdiag_d)

            if do_l2:
                last_pass = plan2["n_passes"] - 1
                with (tc.tile_pool(name="ostage", bufs=4) as osp,
                      tc.tile_pool(name="l2acc", bufs=1) as l2p):
                    agg2 = (l2p.tile([OUT_DIM, SHARD_PAD], DT32)
                            if plan2["n_passes"] > 1 else None)
                    def l2_out(w, ps, pi):
                        t = w * WIN
                        if plan2["n_passes"] == 1:
                            ot = osp.tile([OUT_DIM, WIN], DT32, name="ot")
                            nc.vector.tensor_scalar(
                                out=ot[:], in0=ps[:],
                                scalar1=b2_sb[:, 0:1], scalar2=None,
                                op0=mybir.AluOpType.add)
                            nc.sync.dma_start(out_d[:, t:t + WIN], ot[:])
                            return
                        if pi == 0:
                            nc.vector.tensor_scalar(
                                out=agg2[:, t:t + WIN], in0=ps[:],
                                scalar1=b2_sb[:, 0:1], scalar2=None,
                                op0=mybir.AluOpType.add)
                        elif pi < last_pass:
                            nc.vector.tensor_tensor(
                                out=agg2[:, t:t + WIN], in0=ps[:],
                                in1=agg2[:, t:t + WIN],
                                op=mybir.AluOpType.add)
                        else:
                            ot = osp.tile([OUT_DIM, WIN], DT32, name="ot")
                            nc.vector.tensor_tensor(
                                out=ot[:], in0=ps[:], in1=agg2[:, t:t + WIN],
                                op=mybir.AluOpType.add)
                            nc.sync.dma_start(out_d[:, t:t + WIN], ot[:])
                    _emit_agg_phase(nc, tc, plan2, tables2, idx2_d, dn2_d,
                                    nm2_d, iota_sb, OUT_DIM, l2_out, "g2",
                                    elem2, cfg, diag_d=m_local)
            else:
                with tc.tile_pool(name="ostage", bufs=4) as osp:
                    for w in range(N_WIN):
                        ot = osp.tile([OUT_DIM, WIN], DT32, name="ot")
                        nc.vector.memset(ot[:], 0.0)
                        nc.sync.dma_start(out_d[:, w * WIN:(w + 1) * WIN], ot[:])
    nc.compile()

    x16 = x.astype(np.float16)
    iota = np.tile(np.arange(WIN, dtype=np.float16), (128, 1))
    in_maps = []
    for c in range(N_CORES):
        uniq = plan1["uniq_c"][c]
        xt = np.zeros((uniq_pad, elem1), np.float16)
        xt[:len(uniq), :IN_DIM] = x16[uniq]
        xd = None
        if diag:
            xd = np.zeros((SHARD_PAD, elem1), np.float16)
            xd[:SHARD, :IN_DIM] = x16[c * SHARD:(c + 1) * SHARD]
        in_maps.append({
            **({"xdiag": xd} if diag else {}),
            "xt": xt,
            "idx1": plan1["idx_packed"][c], "idx2": plan2["idx_packed"][c],
            "dn1": plan1["dn"][c], "nm1": plan1["nm"][c],
            "dn2": plan2["dn"][c], "nm2": plan2["nm"][c],
            "w1": w1.astype(np.float16), "w2": w2.astype(np.float16),
            "b1": b1.reshape(-1, 1).astype(np.float32),
            "b2": b2.reshape(-1, 1).astype(np.float32),
            "iota": iota,
        })
    return nc, in_maps


def kernel(x, edge_index, W1, b1, W2, b2, cfg=None):
    x = np.asarray(x); edge_index = np.asarray(edge_index)
    W1 = np.asarray(W1); b1 = np.asarray(b1)
    W2 = np.asarray(W2); b2 = np.asarray(b2)
    nc, in_maps = build_kernel(edge_index, W1, b1, W2, b2, x, cfg=cfg)
    res = run_bass_kernel_spmd(nc, in_maps, core_ids=list(range(N_CORES)))
    out = np.empty((N_NODES, OUT_DIM), np.float32)
    for c in range(N_CORES):
        out[c * SHARD:(c + 1) * SHARD] = res.results[c]["outT"].T[:SHARD]
    return out
